# revision 7
# baseline (speedup 1.0000x reference)
"""Two-level Strassen LoRA-folded Linear on 8 TRN2 NeuronCores.

out[b] = x[b] @ W_eff[e_b].T + bias,  W_eff = W + SCALING * lb @ la  (host fold)

Two recursive Strassen levels: 49 sub-GEMMs of [512 seq, 1024 K, 1024 N] =
49/64 of the classical bf16 PE rows. Input combinations (both levels) are host
marshaling; the device multiplies and combines P-products into 16 bf16 C
quarter-accumulators during the PSUM drain (Act copy + DVE adds), bias folded
into each quarter's first contribution. Output bf16, upcast on host.
Numerically verified recipe: ~1.2e-2 rel err (gate 2e-2).
"""

from contextlib import ExitStack

import ml_dtypes
import numpy as np

SCALING = 32.0 / 16.0
B, S, D_IN, D_OUT, R, E = 8, 2048, 4096, 4096, 16, 8

KT = 128
SQ, KQ, NQ = S // 4, D_IN // 4, D_OUT // 4  # 512, 1024, 1024 quarter sizes

ALPHA = {
    0: {(0, 0): 1, (1, 1): 1}, 1: {(1, 0): 1, (1, 1): 1}, 2: {(0, 0): 1},
    3: {(1, 1): 1}, 4: {(0, 0): 1, (0, 1): 1}, 5: {(1, 0): 1, (0, 0): -1},
    6: {(0, 1): 1, (1, 1): -1},
}
BETA = {
    0: {(0, 0): 1, (1, 1): 1}, 1: {(0, 0): 1}, 2: {(0, 1): 1, (1, 1): -1},
    3: {(1, 0): 1, (0, 0): -1}, 4: {(1, 1): 1}, 5: {(0, 0): 1, (0, 1): 1},
    6: {(1, 0): 1, (1, 1): 1},
}
GAMMA = {
    0: {(0, 0): 1, (1, 1): 1}, 1: {(1, 0): 1, (1, 1): -1}, 2: {(0, 1): 1, (1, 1): 1},
    3: {(0, 0): 1, (1, 0): 1}, 4: {(0, 1): 1, (0, 0): -1}, 5: {(1, 1): 1},
    6: {(0, 0): 1},
}
ORDER = [0, 3, 4, 6, 1, 2, 5]

# execution sequence of the 49 products, host pack order == device order
SEQ = [(i, i2) for i in ORDER for i2 in ORDER]
# per product: list of (quarter_key, sign); quarter_key = (I1, I2, L1, L2)
CONTRIB2 = [
    [
        ((I1, I2, L1, L2), g1 * g2)
        for (I1, L1), g1 in GAMMA[i].items()
        for (I2, L2), g2 in GAMMA[i2].items()
    ]
    for (i, i2) in SEQ
]
_first = {}
_last = {}
for qi, contribs in enumerate(CONTRIB2):
    for key, _sgn in contribs:
        _first.setdefault(key, qi)
        _last[key] = qi
FIRST2, LAST2 = _first, _last


def build_nc(
    passes=1,
    a_bufs=3,
    b_bufs=4,
    c_bufs=16,
    psum_bufs=2,  # [128, 4, 512] tiles = 4 banks each
):
    import concourse.mybir as mybir
    import concourse.tile as tile
    from concourse import bacc

    bf16 = mybir.dt.bfloat16
    f32 = mybir.dt.float32
    add, sub = mybir.AluOpType.add, mybir.AluOpType.subtract

    nc = bacc.Bacc("TRN2", target_bir_lowering=False, debug=False, enable_asserts=False)
    acT = nc.dram_tensor("acT", [49 * KQ, SQ], bf16, kind="ExternalInput").ap()
    bcT = nc.dram_tensor("bcT", [49 * KQ, NQ], bf16, kind="ExternalInput").ap()
    biasB = nc.dram_tensor("biasB", [KT, D_OUT], bf16, kind="ExternalInput").ap()
    out = nc.dram_tensor("out", [S, D_OUT], bf16, kind="ExternalOutput").ap()

    n_kt = KQ // KT  # 8 k-tiles per sub-GEMM

    with tile.TileContext(nc) as tc, ExitStack() as ctx:
        apool = ctx.enter_context(tc.tile_pool(name="a", bufs=a_bufs))
        bpool = ctx.enter_context(tc.tile_pool(name="b", bufs=b_bufs))
        cpool = ctx.enter_context(tc.tile_pool(name="c", bufs=c_bufs))
        bias_pool = ctx.enter_context(tc.tile_pool(name="bias", bufs=1))
        tpool = ctx.enter_context(tc.tile_pool(name="tmp", bufs=2))
        pspool = ctx.enter_context(tc.tile_pool(name="ps", bufs=psum_bufs, space="PSUM"))

        bias_t = bias_pool.tile([KT, D_OUT], bf16, tag="bias", name="biast")
        nc.sync.dma_start(bias_t[:], biasB[:])

        def issue_A(p, qi):
            t = apool.tile([KT, n_kt, SQ], bf16, tag="a", name=f"a{p}_{qi}")
            src = acT[qi * KQ : (qi + 1) * KQ, :]
            nc.sync.dma_start(t[:], src.rearrange("(k p) s -> p k s", p=KT))
            return t

        def issue_B(p, qi, pc):
            t = bpool.tile([KT, n_kt, 512], bf16, tag="b", name=f"b{p}_{qi}_{pc}")
            src = bcT[qi * KQ : (qi + 1) * KQ, pc * 512 : (pc + 1) * 512]
            nc.sync.dma_start(t[:], src.rearrange("(k p) o -> p k o", p=KT))
            return t

        a_pre, b_pre = {}, {}

        def prefetch(key):
            p, qi = key
            if key not in a_pre:
                a_pre[key] = issue_A(p, qi)
                b_pre[key] = [issue_B(p, qi, pc) for pc in range(2)]

        n_q = len(SEQ)
        for p in range(passes):
            c_tiles = {}
            prefetch((p, 0))
            for qi in range(n_q):
                at = a_pre.pop((p, qi))
                bt = b_pre.pop((p, qi))
                if qi + 1 < n_q:
                    prefetch((p, qi + 1))
                elif p + 1 < passes:
                    prefetch((p + 1, 0))
                for key, _sgn in CONTRIB2[qi]:
                    if FIRST2[key] == qi:
                        c_tiles[key] = cpool.tile(
                            [KT, 4, NQ], bf16, tag="c", name=f"c{p}_{key}"
                        )
                for pc in range(2):
                    pts = pspool.tile(
                        [KT, 4, 512], f32, tag="ps", name=f"ps{p}_{qi}_{pc}"
                    )
                    for kt in range(n_kt):
                        for s in range(4):
                            nc.tensor.matmul(
                                pts[:, s, :],
                                at[:, kt, s * KT : (s + 1) * KT],
                                bt[pc][:, kt, :],
                                start=(kt == 0),
                                stop=(kt == n_kt - 1),
                            )
                    tmp = tpool.tile([KT, 4, 512], bf16, tag="tmp", name=f"t{p}_{qi}_{pc}")
                    nc.scalar.copy(tmp[:], pts[:])
                    for key, sgn in CONTRIB2[qi]:
                        ct = c_tiles[key]
                        dstap = ct[:, :, pc * 512 : (pc + 1) * 512]
                        if FIRST2[key] == qi:
                            bcol = key[2] * 2048 + key[3] * 1024 + pc * 512
                            for jj in range(4):
                                nc.vector.tensor_tensor(
                                    ct[:, jj, pc * 512 : (pc + 1) * 512],
                                    tmp[:, jj, :],
                                    bias_t[:, bcol : bcol + 512],
                                    add,
                                )
                        elif sgn > 0:
                            nc.vector.tensor_tensor(dstap, tmp[:], dstap, add)
                        else:
                            nc.vector.tensor_tensor(dstap, dstap, tmp[:], sub)
                for key, _sgn in CONTRIB2[qi]:
                    if LAST2[key] == qi:
                        I1, I2, L1, L2 = key
                        r0 = I1 * 1024 + I2 * 512
                        c0 = L1 * 2048 + L2 * 1024
                        dst = out[r0 : r0 + SQ, c0 : c0 + NQ]
                        nc.sync.dma_start(
                            dst.rearrange("(g q) o -> q g o", q=KT), c_tiles[key][:]
                        )

    nc.compile()
    return nc


def make_in_maps(x, expert_ids, W, b, lora_a, lora_b):
    eids = np.asarray(expert_ids).astype(np.int64)
    W = np.asarray(W, dtype=np.float32)
    bias_bcast = np.ascontiguousarray(
        np.broadcast_to(np.asarray(b, dtype=np.float32)[None, :], (KT, D_OUT))
    ).astype(ml_dtypes.bfloat16)
    bc_cache = {}
    in_maps = []
    for c in range(x.shape[0]):
        e = int(eids[c])
        if e not in bc_cache:
            delta = SCALING * (
                np.asarray(lora_b[e], dtype=np.float32)
                @ np.asarray(lora_a[e], dtype=np.float32)
            )
            Bm = np.ascontiguousarray((W + delta).T)  # [K, N]

            def bblk(J1, J2, L1, L2):
                r = J1 * 2048 + J2 * 1024
                cc = L1 * 2048 + L2 * 1024
                return Bm[r : r + KQ, cc : cc + NQ]

            combos = []
            for i, i2 in SEQ:
                acc = np.zeros((KQ, NQ), np.float32)
                for (J1, L1), s1 in BETA[i].items():
                    for (J2, L2), s2 in BETA[i2].items():
                        acc += (s1 * s2) * bblk(J1, J2, L1, L2)
                combos.append(acc)
            bc_cache[e] = np.concatenate(combos, axis=0).astype(ml_dtypes.bfloat16)
        xc = np.asarray(x[c], dtype=np.float32)

        def ablk(I1, I2, J1, J2):
            r = I1 * 1024 + I2 * 512
            cc = J1 * 2048 + J2 * 1024
            return xc[r : r + SQ, cc : cc + KQ]

        acombos = []
        for i, i2 in SEQ:
            acc = np.zeros((SQ, KQ), np.float32)
            for (I1, J1), s1 in ALPHA[i].items():
                for (I2, J2), s2 in ALPHA[i2].items():
                    acc += (s1 * s2) * ablk(I1, I2, J1, J2)
            acombos.append(acc.T)  # [KQ, SQ]
        ac = np.concatenate(acombos, axis=0).astype(ml_dtypes.bfloat16)
        in_maps.append({"acT": ac, "bcT": bc_cache[e], "biasB": bias_bcast})
    return in_maps


_NC_CACHE = {}


def kernel(x, expert_ids, W, b, lora_a, lora_b):
    from concourse.bass_utils import run_bass_kernel_spmd

    x = np.asarray(x)
    if "nc" not in _NC_CACHE:
        _NC_CACHE["nc"] = build_nc()
    nc = _NC_CACHE["nc"]
    in_maps = make_in_maps(x, expert_ids, W, b, lora_a, lora_b)
    res = run_bass_kernel_spmd(nc, in_maps, core_ids=list(range(B))).results
    return np.stack(
        [res[c]["out"].astype(np.float32) for c in range(B)], axis=0
    )


# revision 8
# speedup vs baseline: 1.0174x; 1.0174x over previous
"""Two-level Strassen LoRA-folded Linear on 8 TRN2 NeuronCores.

out[b] = x[b] @ W_eff[e_b].T + bias,  W_eff = W + SCALING * lb @ la  (host fold)

Two recursive Strassen levels: 49 sub-GEMMs of [512 seq, 1024 K, 1024 N] =
49/64 of the classical bf16 PE rows. Input combinations (both levels) are host
marshaling; the device multiplies and combines P-products into 16 bf16 C
quarter-accumulators during the PSUM drain (Act copy + DVE adds), bias folded
into each quarter's first contribution. Output bf16, upcast on host.
Numerically verified recipe: ~1.2e-2 rel err (gate 2e-2).
"""

from contextlib import ExitStack

import ml_dtypes
import numpy as np

SCALING = 32.0 / 16.0
B, S, D_IN, D_OUT, R, E = 8, 2048, 4096, 4096, 16, 8

KT = 128
SQ, KQ, NQ = S // 4, D_IN // 4, D_OUT // 4  # 512, 1024, 1024 quarter sizes

ALPHA = {
    0: {(0, 0): 1, (1, 1): 1}, 1: {(1, 0): 1, (1, 1): 1}, 2: {(0, 0): 1},
    3: {(1, 1): 1}, 4: {(0, 0): 1, (0, 1): 1}, 5: {(1, 0): 1, (0, 0): -1},
    6: {(0, 1): 1, (1, 1): -1},
}
BETA = {
    0: {(0, 0): 1, (1, 1): 1}, 1: {(0, 0): 1}, 2: {(0, 1): 1, (1, 1): -1},
    3: {(1, 0): 1, (0, 0): -1}, 4: {(1, 1): 1}, 5: {(0, 0): 1, (0, 1): 1},
    6: {(1, 0): 1, (1, 1): 1},
}
GAMMA = {
    0: {(0, 0): 1, (1, 1): 1}, 1: {(1, 0): 1, (1, 1): -1}, 2: {(0, 1): 1, (1, 1): 1},
    3: {(0, 0): 1, (1, 0): 1}, 4: {(0, 1): 1, (0, 0): -1}, 5: {(1, 1): 1},
    6: {(0, 0): 1},
}
# P4, P7, P1, P5, P2, P3, P6: spreads each C-region's last contribution away
# from the pass end so quarter flushes get >=17 units of lead before the next
# pass reuses their SBUF buffers; every region's first contribution stays +.
ORDER = [3, 6, 0, 4, 1, 2, 5]

# execution sequence of the 49 products, host pack order == device order
SEQ = [(i, i2) for i in ORDER for i2 in ORDER]
# per product: list of (quarter_key, sign); quarter_key = (I1, I2, L1, L2)
CONTRIB2 = [
    [
        ((I1, I2, L1, L2), g1 * g2)
        for (I1, L1), g1 in GAMMA[i].items()
        for (I2, L2), g2 in GAMMA[i2].items()
    ]
    for (i, i2) in SEQ
]
_first = {}
_last = {}
_first_sign = {}
for qi, contribs in enumerate(CONTRIB2):
    for key, sgn in contribs:
        if key not in _first:
            _first[key] = qi
            _first_sign[key] = sgn
        _last[key] = qi
FIRST2, LAST2 = _first, _last
assert all(s > 0 for s in _first_sign.values()), "bias-on-first needs + first sign"


def build_nc(
    passes=1,
    a_bufs=3,
    b_bufs=4,
    c_bufs=16,
    psum_bufs=2,  # [128, 4, 512] tiles = 4 banks each
):
    import concourse.mybir as mybir
    import concourse.tile as tile
    from concourse import bacc

    bf16 = mybir.dt.bfloat16
    f32 = mybir.dt.float32
    add, sub = mybir.AluOpType.add, mybir.AluOpType.subtract

    nc = bacc.Bacc("TRN2", target_bir_lowering=False, debug=False, enable_asserts=False)
    acT = nc.dram_tensor("acT", [49 * KQ, SQ], bf16, kind="ExternalInput").ap()
    bcT = nc.dram_tensor("bcT", [49 * KQ, NQ], bf16, kind="ExternalInput").ap()
    biasB = nc.dram_tensor("biasB", [KT, D_OUT], bf16, kind="ExternalInput").ap()
    out = nc.dram_tensor("out", [S, D_OUT], bf16, kind="ExternalOutput").ap()

    n_kt = KQ // KT  # 8 k-tiles per sub-GEMM

    with tile.TileContext(nc) as tc, ExitStack() as ctx:
        apool = ctx.enter_context(tc.tile_pool(name="a", bufs=a_bufs))
        bpool = ctx.enter_context(tc.tile_pool(name="b", bufs=b_bufs))
        cpool = ctx.enter_context(tc.tile_pool(name="c", bufs=c_bufs))
        bias_pool = ctx.enter_context(tc.tile_pool(name="bias", bufs=1))
        tpool = ctx.enter_context(tc.tile_pool(name="tmp", bufs=2))
        pspool = ctx.enter_context(tc.tile_pool(name="ps", bufs=psum_bufs, space="PSUM"))

        bias_t = bias_pool.tile([KT, D_OUT], bf16, tag="bias", name="biast")
        nc.sync.dma_start(bias_t[:], biasB[:])

        def issue_A(p, qi):
            t = apool.tile([KT, n_kt, SQ], bf16, tag="a", name=f"a{p}_{qi}")
            src = acT[qi * KQ : (qi + 1) * KQ, :]
            nc.sync.dma_start(t[:], src.rearrange("(k p) s -> p k s", p=KT))
            return t

        def issue_B(p, qi, pc):
            t = bpool.tile([KT, n_kt, 512], bf16, tag="b", name=f"b{p}_{qi}_{pc}")
            src = bcT[qi * KQ : (qi + 1) * KQ, pc * 512 : (pc + 1) * 512]
            nc.sync.dma_start(t[:], src.rearrange("(k p) o -> p k o", p=KT))
            return t

        a_pre, b_pre = {}, {}

        def prefetch(key):
            p, qi = key
            if key not in a_pre:
                a_pre[key] = issue_A(p, qi)
                b_pre[key] = [issue_B(p, qi, pc) for pc in range(2)]

        n_q = len(SEQ)
        for p in range(passes):
            c_tiles = {}
            prefetch((p, 0))
            for qi in range(n_q):
                at = a_pre.pop((p, qi))
                bt = b_pre.pop((p, qi))
                if qi + 1 < n_q:
                    prefetch((p, qi + 1))
                elif p + 1 < passes:
                    prefetch((p + 1, 0))
                for key, _sgn in CONTRIB2[qi]:
                    if FIRST2[key] == qi:
                        c_tiles[key] = cpool.tile(
                            [KT, 4, NQ], bf16, tag="c", name=f"c{p}_{key}"
                        )
                for pc in range(2):
                    pts = pspool.tile(
                        [KT, 4, 512], f32, tag="ps", name=f"ps{p}_{qi}_{pc}"
                    )
                    for kt in range(n_kt):
                        for s in range(4):
                            nc.tensor.matmul(
                                pts[:, s, :],
                                at[:, kt, s * KT : (s + 1) * KT],
                                bt[pc][:, kt, :],
                                start=(kt == 0),
                                stop=(kt == n_kt - 1),
                            )
                    tmp = tpool.tile([KT, 4, 512], bf16, tag="tmp", name=f"t{p}_{qi}_{pc}")
                    nc.scalar.copy(tmp[:], pts[:])
                    for key, sgn in CONTRIB2[qi]:
                        ct = c_tiles[key]
                        dstap = ct[:, :, pc * 512 : (pc + 1) * 512]
                        if FIRST2[key] == qi:
                            bcol = key[2] * 2048 + key[3] * 1024 + pc * 512
                            for jj in range(4):
                                nc.vector.tensor_tensor(
                                    ct[:, jj, pc * 512 : (pc + 1) * 512],
                                    tmp[:, jj, :],
                                    bias_t[:, bcol : bcol + 512],
                                    add,
                                )
                        elif sgn > 0:
                            nc.vector.tensor_tensor(dstap, tmp[:], dstap, add)
                        else:
                            nc.vector.tensor_tensor(dstap, dstap, tmp[:], sub)
                for key, _sgn in CONTRIB2[qi]:
                    if LAST2[key] == qi:
                        I1, I2, L1, L2 = key
                        r0 = I1 * 1024 + I2 * 512
                        c0 = L1 * 2048 + L2 * 1024
                        dst = out[r0 : r0 + SQ, c0 : c0 + NQ]
                        nc.sync.dma_start(
                            dst.rearrange("(g q) o -> q g o", q=KT), c_tiles[key][:]
                        )

    nc.compile()
    return nc


def make_in_maps(x, expert_ids, W, b, lora_a, lora_b):
    eids = np.asarray(expert_ids).astype(np.int64)
    W = np.asarray(W, dtype=np.float32)
    bias_bcast = np.ascontiguousarray(
        np.broadcast_to(np.asarray(b, dtype=np.float32)[None, :], (KT, D_OUT))
    ).astype(ml_dtypes.bfloat16)
    bc_cache = {}
    in_maps = []
    for c in range(x.shape[0]):
        e = int(eids[c])
        if e not in bc_cache:
            delta = SCALING * (
                np.asarray(lora_b[e], dtype=np.float32)
                @ np.asarray(lora_a[e], dtype=np.float32)
            )
            Bm = np.ascontiguousarray((W + delta).T)  # [K, N]

            def bblk(J1, J2, L1, L2):
                r = J1 * 2048 + J2 * 1024
                cc = L1 * 2048 + L2 * 1024
                return Bm[r : r + KQ, cc : cc + NQ]

            combos = []
            for i, i2 in SEQ:
                acc = np.zeros((KQ, NQ), np.float32)
                for (J1, L1), s1 in BETA[i].items():
                    for (J2, L2), s2 in BETA[i2].items():
                        acc += (s1 * s2) * bblk(J1, J2, L1, L2)
                combos.append(acc)
            bc_cache[e] = np.concatenate(combos, axis=0).astype(ml_dtypes.bfloat16)
        xc = np.asarray(x[c], dtype=np.float32)

        def ablk(I1, I2, J1, J2):
            r = I1 * 1024 + I2 * 512
            cc = J1 * 2048 + J2 * 1024
            return xc[r : r + SQ, cc : cc + KQ]

        acombos = []
        for i, i2 in SEQ:
            acc = np.zeros((SQ, KQ), np.float32)
            for (I1, J1), s1 in ALPHA[i].items():
                for (I2, J2), s2 in ALPHA[i2].items():
                    acc += (s1 * s2) * ablk(I1, I2, J1, J2)
            acombos.append(acc.T)  # [KQ, SQ]
        ac = np.concatenate(acombos, axis=0).astype(ml_dtypes.bfloat16)
        in_maps.append({"acT": ac, "bcT": bc_cache[e], "biasB": bias_bcast})
    return in_maps


_NC_CACHE = {}


def kernel(x, expert_ids, W, b, lora_a, lora_b):
    from concourse.bass_utils import run_bass_kernel_spmd

    x = np.asarray(x)
    if "nc" not in _NC_CACHE:
        _NC_CACHE["nc"] = build_nc()
    nc = _NC_CACHE["nc"]
    in_maps = make_in_maps(x, expert_ids, W, b, lora_a, lora_b)
    res = run_bass_kernel_spmd(nc, in_maps, core_ids=list(range(B))).results
    return np.stack(
        [res[c]["out"].astype(np.float32) for c in range(B)], axis=0
    )


# revision 9
# speedup vs baseline: 1.0524x; 1.0343x over previous
"""Two-level Strassen LoRA-folded Linear on 8 TRN2 NeuronCores.

out[b] = x[b] @ W_eff[e_b].T + bias,  W_eff = W + SCALING * lb @ la  (host fold)

Two recursive Strassen levels: 49 sub-GEMMs of [512 seq, 1024 K, 1024 N] =
49/64 of the classical bf16 PE rows. Input combinations (both levels) are host
marshaling; the device multiplies and combines P-products into 16 bf16 C
quarter-accumulators during the PSUM drain (Act copy + DVE adds), bias folded
into each quarter's first contribution. Output bf16, upcast on host.
Numerically verified recipe: ~1.2e-2 rel err (gate 2e-2).
"""

from contextlib import ExitStack

import ml_dtypes
import numpy as np

SCALING = 32.0 / 16.0
B, S, D_IN, D_OUT, R, E = 8, 2048, 4096, 4096, 16, 8

KT = 128
SQ, KQ, NQ = S // 4, D_IN // 4, D_OUT // 4  # 512, 1024, 1024 quarter sizes

ALPHA = {
    0: {(0, 0): 1, (1, 1): 1}, 1: {(1, 0): 1, (1, 1): 1}, 2: {(0, 0): 1},
    3: {(1, 1): 1}, 4: {(0, 0): 1, (0, 1): 1}, 5: {(1, 0): 1, (0, 0): -1},
    6: {(0, 1): 1, (1, 1): -1},
}
BETA = {
    0: {(0, 0): 1, (1, 1): 1}, 1: {(0, 0): 1}, 2: {(0, 1): 1, (1, 1): -1},
    3: {(1, 0): 1, (0, 0): -1}, 4: {(1, 1): 1}, 5: {(0, 0): 1, (0, 1): 1},
    6: {(1, 0): 1, (1, 1): 1},
}
GAMMA = {
    0: {(0, 0): 1, (1, 1): 1}, 1: {(1, 0): 1, (1, 1): -1}, 2: {(0, 1): 1, (1, 1): 1},
    3: {(0, 0): 1, (1, 0): 1}, 4: {(0, 1): 1, (0, 0): -1}, 5: {(1, 1): 1},
    6: {(0, 0): 1},
}
# P4, P7, P1, P5, P2, P3, P6: spreads each C-region's last contribution away
# from the pass end so quarter flushes get >=17 units of lead before the next
# pass reuses their SBUF buffers; every region's first contribution stays +.
ORDER = [3, 6, 0, 4, 1, 2, 5]

# execution sequence of the 49 products, host pack order == device order
SEQ = [(i, i2) for i in ORDER for i2 in ORDER]
# per product: list of (quarter_key, sign); quarter_key = (I1, I2, L1, L2)
CONTRIB2 = [
    [
        ((I1, I2, L1, L2), g1 * g2)
        for (I1, L1), g1 in GAMMA[i].items()
        for (I2, L2), g2 in GAMMA[i2].items()
    ]
    for (i, i2) in SEQ
]
_first = {}
_last = {}
_first_sign = {}
for qi, contribs in enumerate(CONTRIB2):
    for key, sgn in contribs:
        if key not in _first:
            _first[key] = qi
            _first_sign[key] = sgn
        _last[key] = qi
FIRST2, LAST2 = _first, _last
assert all(s > 0 for s in _first_sign.values()), "bias-on-first needs + first sign"


def build_nc(
    passes=1,
    a_bufs=3,
    b_bufs=3,
    c_bufs=16,
    psum_bufs=2,  # [128, 4, 512] tiles = 4 banks each
):
    import concourse.mybir as mybir
    import concourse.tile as tile
    from concourse import bacc

    bf16 = mybir.dt.bfloat16
    f32 = mybir.dt.float32
    add, sub = mybir.AluOpType.add, mybir.AluOpType.subtract

    nc = bacc.Bacc("TRN2", target_bir_lowering=False, debug=False, enable_asserts=False)
    acT = nc.dram_tensor("acT", [49 * KQ, SQ], bf16, kind="ExternalInput").ap()
    bcT = nc.dram_tensor("bcT", [49 * KQ, NQ], bf16, kind="ExternalInput").ap()
    biasB = nc.dram_tensor("biasB", [KT, D_OUT], bf16, kind="ExternalInput").ap()
    out = nc.dram_tensor("out", [S, D_OUT], bf16, kind="ExternalOutput").ap()

    n_kt = KQ // KT  # 8 k-tiles per sub-GEMM

    with tile.TileContext(nc) as tc, ExitStack() as ctx:
        apool = ctx.enter_context(tc.tile_pool(name="a", bufs=a_bufs))
        bpool = ctx.enter_context(tc.tile_pool(name="b", bufs=b_bufs))
        cpool = ctx.enter_context(tc.tile_pool(name="c", bufs=c_bufs))
        bias_pool = ctx.enter_context(tc.tile_pool(name="bias", bufs=1))
        tpool = ctx.enter_context(tc.tile_pool(name="tmp", bufs=2))
        pspool = ctx.enter_context(tc.tile_pool(name="ps", bufs=psum_bufs, space="PSUM"))

        bias_t = bias_pool.tile([KT, D_OUT], bf16, tag="bias", name="biast")
        nc.sync.dma_start(bias_t[:], biasB[:])

        def issue_A(p, qi):
            t = apool.tile([KT, n_kt, SQ], bf16, tag="a", name=f"a{p}_{qi}")
            src = acT[qi * KQ : (qi + 1) * KQ, :]
            nc.sync.dma_start(t[:], src.rearrange("(k p) s -> p k s", p=KT))
            return t

        def issue_B(p, qi, pc):
            t = bpool.tile([KT, n_kt, 512], bf16, tag="b", name=f"b{p}_{qi}_{pc}")
            src = bcT[qi * KQ : (qi + 1) * KQ, pc * 512 : (pc + 1) * 512]
            nc.sync.dma_start(t[:], src.rearrange("(k p) o -> p k o", p=KT))
            return t

        a_pre, b_pre = {}, {}

        def prefetch(key):
            p, qi = key
            if key not in a_pre:
                a_pre[key] = issue_A(p, qi)
                b_pre[key] = [issue_B(p, qi, pc) for pc in range(2)]

        n_q = len(SEQ)
        for p in range(passes):
            c_tiles = {}
            prefetch((p, 0))
            for qi in range(n_q):
                at = a_pre.pop((p, qi))
                bt = b_pre.pop((p, qi))
                if qi + 1 < n_q:
                    prefetch((p, qi + 1))
                elif p + 1 < passes:
                    prefetch((p + 1, 0))
                for key, _sgn in CONTRIB2[qi]:
                    if FIRST2[key] == qi:
                        c_tiles[key] = cpool.tile(
                            [KT, 4, NQ], bf16, tag="c", name=f"c{p}_{key}"
                        )
                # one full-width tmp per product: Act fills each pc half as its
                # PSUM unit finishes; DVE then combines [128, 4, 1024] in one
                # op per contribution (half the dispatch/semaphore traffic)
                tmp = tpool.tile([KT, 4, NQ], bf16, tag="tmp", name=f"t{p}_{qi}")
                for pc in range(2):
                    pts = pspool.tile(
                        [KT, 4, 512], f32, tag="ps", name=f"ps{p}_{qi}_{pc}"
                    )
                    for kt in range(n_kt):
                        for s in range(4):
                            nc.tensor.matmul(
                                pts[:, s, :],
                                at[:, kt, s * KT : (s + 1) * KT],
                                bt[pc][:, kt, :],
                                start=(kt == 0),
                                stop=(kt == n_kt - 1),
                            )
                    nc.scalar.copy(tmp[:, :, pc * 512 : (pc + 1) * 512], pts[:])
                for key, sgn in CONTRIB2[qi]:
                    ct = c_tiles[key]
                    if FIRST2[key] == qi:
                        bcol = key[2] * 2048 + key[3] * 1024
                        for jj in range(4):
                            nc.vector.tensor_tensor(
                                ct[:, jj, :],
                                tmp[:, jj, :],
                                bias_t[:, bcol : bcol + NQ],
                                add,
                            )
                    elif sgn > 0:
                        nc.vector.tensor_tensor(ct[:], tmp[:], ct[:], add)
                    else:
                        nc.vector.tensor_tensor(ct[:], ct[:], tmp[:], sub)
                for key, _sgn in CONTRIB2[qi]:
                    if LAST2[key] == qi:
                        I1, I2, L1, L2 = key
                        r0 = I1 * 1024 + I2 * 512
                        c0 = L1 * 2048 + L2 * 1024
                        dst = out[r0 : r0 + SQ, c0 : c0 + NQ]
                        nc.sync.dma_start(
                            dst.rearrange("(g q) o -> q g o", q=KT), c_tiles[key][:]
                        )

    nc.compile()
    return nc


def make_in_maps(x, expert_ids, W, b, lora_a, lora_b):
    eids = np.asarray(expert_ids).astype(np.int64)
    W = np.asarray(W, dtype=np.float32)
    bias_bcast = np.ascontiguousarray(
        np.broadcast_to(np.asarray(b, dtype=np.float32)[None, :], (KT, D_OUT))
    ).astype(ml_dtypes.bfloat16)
    bc_cache = {}
    in_maps = []
    for c in range(x.shape[0]):
        e = int(eids[c])
        if e not in bc_cache:
            delta = SCALING * (
                np.asarray(lora_b[e], dtype=np.float32)
                @ np.asarray(lora_a[e], dtype=np.float32)
            )
            Bm = np.ascontiguousarray((W + delta).T)  # [K, N]

            def bblk(J1, J2, L1, L2):
                r = J1 * 2048 + J2 * 1024
                cc = L1 * 2048 + L2 * 1024
                return Bm[r : r + KQ, cc : cc + NQ]

            combos = []
            for i, i2 in SEQ:
                acc = np.zeros((KQ, NQ), np.float32)
                for (J1, L1), s1 in BETA[i].items():
                    for (J2, L2), s2 in BETA[i2].items():
                        acc += (s1 * s2) * bblk(J1, J2, L1, L2)
                combos.append(acc)
            bc_cache[e] = np.concatenate(combos, axis=0).astype(ml_dtypes.bfloat16)
        xc = np.asarray(x[c], dtype=np.float32)

        def ablk(I1, I2, J1, J2):
            r = I1 * 1024 + I2 * 512
            cc = J1 * 2048 + J2 * 1024
            return xc[r : r + SQ, cc : cc + KQ]

        acombos = []
        for i, i2 in SEQ:
            acc = np.zeros((SQ, KQ), np.float32)
            for (I1, J1), s1 in ALPHA[i].items():
                for (I2, J2), s2 in ALPHA[i2].items():
                    acc += (s1 * s2) * ablk(I1, I2, J1, J2)
            acombos.append(acc.T)  # [KQ, SQ]
        ac = np.concatenate(acombos, axis=0).astype(ml_dtypes.bfloat16)
        in_maps.append({"acT": ac, "bcT": bc_cache[e], "biasB": bias_bcast})
    return in_maps


_NC_CACHE = {}


def kernel(x, expert_ids, W, b, lora_a, lora_b):
    from concourse.bass_utils import run_bass_kernel_spmd

    x = np.asarray(x)
    if "nc" not in _NC_CACHE:
        _NC_CACHE["nc"] = build_nc()
    nc = _NC_CACHE["nc"]
    in_maps = make_in_maps(x, expert_ids, W, b, lora_a, lora_b)
    res = run_bass_kernel_spmd(nc, in_maps, core_ids=list(range(B))).results
    return np.stack(
        [res[c]["out"].astype(np.float32) for c in range(B)], axis=0
    )
